# revision 4
# baseline (speedup 1.0000x reference)
"""Batchelor GPU-NUFFT forward operator on 8 Trainium2 NeuronCores.

Math (per timepoint t):
    warped  = bilinear_warp(image, flow[..., t])
    coil    = csm * warped                                  [Nc,Nx,Ny]
    out_t[c,s] = sum_{x,y} coil[c,x,y] exp(-2pi i (kx_s (x-64) + ky_s (y-64)))
    out     = sum_t out_t                                   [Nc,NS] complex64

Sharding: 8 cores = 4 timepoints x 2 sample-halves (4096 samples each).
Host unshard: sum the 4 timepoint partials per half, concat halves.

Per-core structure (the gather overlaps the trig pipeline instead of
serializing in front of it, and the PE drains ramped and back-to-back):
  * warp: interleaved corner table img8[x,y,8] in DRAM, 128 per-column
    [P,1] indirect DMAs on the Pool engine (the only index form the HW
    SWDGE handles), bilinear combine + coil + stationary pack afterwards.
  * NUFFT: Khatri-Rao split y = 64h + 4*yo' + yi (YI=4, two 16-yo halves).
    Moving cos/sin tiles for ALL chunks are computed during the gather
    window (DVE args, +-2^23 round-trick range reduction; scalar Sin
    table, wide 4096-col ACTs) and stored bf16.  Stationary coil packs
    in bf16.  The tail runs 36 matmuls/chunk back-to-back, the outer
    factor A applied from PSUM on DVE, yo-reduction via accumulating
    selector matmuls.
"""

import sys

if "/opt/trn_rl_repo" not in sys.path:
    sys.path.insert(0, "/opt/trn_rl_repo")

import math

import numpy as np

import concourse.bass as bass
import concourse.tile as tile
from concourse import bacc
from concourse import mybir

P = 128
NX = 128
NCOIL = 8
NS = 8192
NT = 4
S = 4096   # samples per core (half of NS)
CH = 512   # samples per PE chunk (one PSUM bank)
CP = 1024  # samples per elementwise chunk-pair
NCHUNK = S // CH
NPAIR = S // CP
YI = 4
YO2 = 16   # yo' per half; YO = 32 total

F32 = mybir.dt.float32
F16 = mybir.dt.float16
BF16 = mybir.dt.bfloat16
I32 = mybir.dt.int32
TWO_PI = float(2.0 * math.pi)
MAGIC = 12582912.0  # 1.5*2^23: (x + M) - M == round-to-nearest(x)
ALU = mybir.AluOpType
ACTF = mybir.ActivationFunctionType


def build_program(nc: bass.Bass, dbg: bool = False):
    def dbg_out(name, src_ap, shape, dtype=F32):
        if not dbg:
            return
        d = nc.dram_tensor("dbg_" + name, shape, dtype, kind="ExternalOutput").ap()
        nc.sync.dma_start(d[:], src_ap)

    image_r = nc.dram_tensor("image_r", [NX, NX], F32, kind="ExternalInput").ap()
    image_i = nc.dram_tensor("image_i", [NX, NX], F32, kind="ExternalInput").ap()
    csm_r = nc.dram_tensor("csm_r", [NCOIL, NX, NX], F32, kind="ExternalInput").ap()
    csm_i = nc.dram_tensor("csm_i", [NCOIL, NX, NX], F32, kind="ExternalInput").ap()
    kx_d = nc.dram_tensor("kx", [S], F32, kind="ExternalInput").ap()
    ky_d = nc.dram_tensor("ky", [S], F32, kind="ExternalInput").ap()
    flow0_d = nc.dram_tensor("flow0", [NX, NX], F32, kind="ExternalInput").ap()
    flow1_d = nc.dram_tensor("flow1", [NX, NX], F32, kind="ExternalInput").ap()
    out_r = nc.dram_tensor("out_r", [NCOIL, S], F32, kind="ExternalOutput").ap()
    out_i = nc.dram_tensor("out_i", [NCOIL, S], F32, kind="ExternalOutput").ap()
    img8_d = nc.dram_tensor("img8_scratch", [NX * NX, 8], F32, kind="Internal").ap()

    # ---------------- inline constants ----------------
    pvals = np.arange(P, dtype=np.float32)
    iota_pf_d = nc.inline_tensor(pvals.reshape(P, 1), name="c_iota_pf").ap()
    xc_d = nc.inline_tensor((pvals - 64.0).reshape(P, 1), name="c_xc").ap()
    yo4_d = nc.inline_tensor((4.0 * (np.arange(P) % YO2)).astype(np.float32)
                             .reshape(P, 1), name="c_yo4").ap()
    half_pi_d = nc.inline_tensor(np.full((P, 1), math.pi / 2, np.float32),
                                 name="c_half_pi").ap()
    jrow_d = nc.inline_tensor(np.tile(np.arange(NX, dtype=np.float32), (P, 1)),
                              name="c_jrow").ap()
    sel_np = (np.arange(P)[:, None] // YO2 == np.arange(NCOIL)[None, :]).astype(
        np.float32)
    sel_d = nc.inline_tensor(sel_np, name="c_sel").ap()

    with tile.TileContext(nc) as tc, \
         tc.tile_pool(name="pp", bufs=1) as pp, \
         tc.tile_pool(name="big", bufs=1) as bp, \
         tc.tile_pool(name="lp", bufs=1) as lp, \
         tc.tile_pool(name="ps", bufs=1, space="PSUM") as ps, \
         tc.tile_pool(name="pso", bufs=1, space="PSUM") as pso:

        # ---- constants ----
        iota_pf = pp.tile([P, 1], F32)
        nc.sync.dma_start(iota_pf[:], iota_pf_d[:])
        xc_col = pp.tile([P, 1], F32)
        nc.sync.dma_start(xc_col[:], xc_d[:])
        yo4 = pp.tile([P, 1], F32)
        nc.sync.dma_start(yo4[:], yo4_d[:])
        half_pi = pp.tile([P, 1], F32)
        nc.sync.dma_start(half_pi[:], half_pi_d[:])
        jrow = pp.tile([P, NX], F32)
        nc.sync.dma_start(jrow[:], jrow_d[:])
        self32 = pp.tile([P, NCOIL], F32)
        nc.sync.dma_start(self32[:], sel_d[:])
        sel = pp.tile([P, NCOIL], BF16)
        nc.vector.tensor_copy(sel[:], self32[:])

        # persistent trig products
        kri = bp.tile([P, NPAIR, YI, CP], BF16)   # -sin(theta_in)
        krr = bp.tile([P, NPAIR, YI, CP], BF16)   # cos(theta_in)
        m2A = bp.tile([P, NPAIR, 2, CP], F16)     # A-factor reduced args
        RA = bp.tile([P, YI, 2, 2, 128], BF16)    # kri-multiplier [-Im | Re]
        RB = bp.tile([P, YI, 2, 2, 128], BF16)    # krr-multiplier [ Re | Im]

        with tc.tile_pool(name="wp", bufs=1) as wp:
            # ================ warp index math + table + gather ================
            fl0 = wp.tile([P, NX], F32)
            nc.sync.dma_start(fl0[:], flow0_d[:])
            fl1 = wp.tile([P, NX], F32)
            nc.sync.dma_start(fl1[:], flow1_d[:])
            img_r_sb = wp.tile([P, NX], F32)
            nc.sync.dma_start(img_r_sb[:], image_r[:])
            img_i_sb = wp.tile([P, NX], F32)
            nc.sync.dma_start(img_i_sb[:], image_i[:])

            cx = wp.tile([P, NX], F32)
            nc.vector.tensor_scalar(cx[:], fl0[:], iota_pf[:, 0:1], None,
                                    op0=ALU.add)
            cx2 = wp.tile([P, NX], F32)
            nc.vector.tensor_scalar(cx2[:], cx[:], 127.0, 0.0, op0=ALU.min,
                                    op1=ALU.max)
            cyt = wp.tile([P, NX], F32)
            nc.vector.tensor_tensor(cyt[:], fl1[:], jrow[:], op=ALU.add)
            cy2 = wp.tile([P, NX], F32)
            nc.vector.tensor_scalar(cy2[:], cyt[:], 127.0, 0.0, op0=ALU.min,
                                    op1=ALU.max)

            c5x = wp.tile([P, NX], F32)
            nc.vector.tensor_scalar(c5x[:], cx2[:], 0.5, None, op0=ALU.subtract)
            x0 = wp.tile([P, NX], F32)
            nc.vector.tensor_scalar(x0[:], c5x[:], MAGIC, MAGIC,
                                    op0=ALU.add, op1=ALU.subtract)
            wx = wp.tile([P, NX], F32)
            nc.vector.tensor_tensor(wx[:], cx2[:], x0[:], op=ALU.subtract)
            c5y = wp.tile([P, NX], F32)
            nc.vector.tensor_scalar(c5y[:], cy2[:], 0.5, None, op0=ALU.subtract)
            y0 = wp.tile([P, NX], F32)
            nc.vector.tensor_scalar(y0[:], c5y[:], MAGIC, MAGIC,
                                    op0=ALU.add, op1=ALU.subtract)
            wy = wp.tile([P, NX], F32)
            nc.vector.tensor_tensor(wy[:], cy2[:], y0[:], op=ALU.subtract)

            idxf = wp.tile([P, NX], F32)
            nc.vector.tensor_scalar(idxf[:], x0[:], 128.0, None, op0=ALU.mult)
            idxf2 = wp.tile([P, NX], F32)
            nc.vector.tensor_tensor(idxf2[:], idxf[:], y0[:], op=ALU.add)
            idx_i = wp.tile([P, NX], I32)
            nc.vector.tensor_copy(idx_i[:], idxf2[:])

            imgBr = wp.tile([P, NX], F32)
            nc.sync.dma_start(imgBr[0:127, :], img_r_sb[1:128, :])
            nc.sync.dma_start(imgBr[127:128, :], img_r_sb[127:128, :])
            imgBi = wp.tile([P, NX], F32)
            nc.sync.dma_start(imgBi[0:127, :], img_i_sb[1:128, :])
            nc.sync.dma_start(imgBi[127:128, :], img_i_sb[127:128, :])

            img8 = wp.tile([P, NX, 8], F32)
            for k, src in ((0, img_r_sb), (2, imgBr), (4, img_i_sb), (6, imgBi)):
                nc.vector.tensor_copy(img8[:, :, k], src[:])
                nc.vector.tensor_copy(img8[:, 0:127, k + 1], src[:, 1:128])
                nc.vector.tensor_copy(img8[:, 127:128, k + 1], src[:, 127:128])
            nc.sync.dma_start(
                img8_d.rearrange("(x y) k -> x (y k)", x=NX), img8[:])

            g8 = wp.tile([P, NX, 8], F32)
            for j in range(NX):
                nc.gpsimd.indirect_dma_start(
                    out=g8[:, j, :],
                    out_offset=None,
                    in_=img8_d[:],
                    in_offset=bass.IndirectOffsetOnAxis(ap=idx_i[:, j:j + 1],
                                                        axis=0),
                )

            # ================ trig pipeline (overlaps the gather) ============
            for cp in range(NPAIR):
                c0 = cp * CP
                kxc = lp.tile([P, CP], F32, tag="kxc")
                nc.sync.dma_start(
                    kxc[:], kx_d[c0:c0 + CP].rearrange(
                        "(p s) -> p s", p=1).to_broadcast([P, CP]))
                kyc = lp.tile([P, CP], F32, tag="kyc")
                nc.sync.dma_start(
                    kyc[:], ky_d[c0:c0 + CP].rearrange(
                        "(p s) -> p s", p=1).to_broadcast([P, CP]))

                u = lp.tile([P, CP], F32, tag="u")
                nc.vector.tensor_scalar(u[:], kxc[:], xc_col[:, 0:1], None,
                                        op0=ALU.mult)
                b64 = lp.tile([P, CP], F32, tag="b64")
                nc.vector.tensor_scalar(b64[:], kyc[:], -64.0, None,
                                        op0=ALU.mult)

                m2p = lp.tile([P, YI, CP], F32, tag="m2p")
                prev = None
                for yi in range(YI):
                    vt = lp.tile([P, CP], F32, tag="vt", bufs=2)
                    if yi == 0:
                        nc.vector.tensor_tensor(vt[:], u[:], b64[:], op=ALU.add)
                    else:
                        nc.vector.tensor_tensor(vt[:], prev[:], kyc[:],
                                                op=ALU.add)
                    r = lp.tile([P, CP], F32, tag="rk")
                    nc.vector.tensor_scalar(r[:], vt[:], MAGIC, MAGIC,
                                            op0=ALU.add, op1=ALU.subtract)
                    nc.vector.tensor_tensor(m2p[:, yi], vt[:], r[:],
                                            op=ALU.subtract)
                    prev = vt

                # kri then in-place Abs then krr (all on scalar, wide ACTs)
                nc.scalar.activation(kri[:, cp], m2p[:], ACTF.Sin,
                                     scale=-TWO_PI)
                nc.scalar.activation(m2p[:], m2p[:], ACTF.Abs)
                nc.scalar.activation(krr[:, cp], m2p[:], ACTF.Sin,
                                     scale=-TWO_PI, bias=half_pi[:, 0:1])

                # A-factor args: vA_h = ky*(4*yo') + 64h*ky
                vA = lp.tile([P, 2, CP], F32, tag="vA")
                nc.vector.tensor_scalar(vA[:, 0], kyc[:], yo4[:, 0:1], None,
                                        op0=ALU.mult)
                k64 = lp.tile([P, CP], F32, tag="k64")
                nc.vector.tensor_scalar(k64[:], kyc[:], 64.0, None,
                                        op0=ALU.mult)
                nc.vector.tensor_tensor(vA[:, 1], vA[:, 0], k64[:], op=ALU.add)
                for h in range(2):
                    rA = lp.tile([P, CP], F32, tag="rA")
                    nc.vector.tensor_scalar(rA[:], vA[:, h], MAGIC, MAGIC,
                                            op0=ALU.add, op1=ALU.subtract)
                    nc.vector.tensor_tensor(m2A[:, cp, h], vA[:, h], rA[:],
                                            op=ALU.subtract)

            # ================ combine + coil + pack (after gather) ==========
            onemwx = wp.tile([P, NX], F32)
            nc.vector.tensor_scalar(onemwx[:], wx[:], -1.0, 1.0, op0=ALU.mult,
                                    op1=ALU.add)
            onemwy = wp.tile([P, NX], F32)
            nc.vector.tensor_scalar(onemwy[:], wy[:], -1.0, 1.0, op0=ALU.mult,
                                    op1=ALU.add)
            w4 = wp.tile([P, NX, 4], F32)
            nc.vector.tensor_tensor(w4[:, :, 0], onemwx[:], onemwy[:],
                                    op=ALU.mult)
            nc.vector.tensor_tensor(w4[:, :, 1], onemwx[:], wy[:], op=ALU.mult)
            nc.vector.tensor_tensor(w4[:, :, 2], wx[:], onemwy[:], op=ALU.mult)
            nc.vector.tensor_tensor(w4[:, :, 3], wx[:], wy[:], op=ALU.mult)

            warped_r = wp.tile([P, NX], F32)
            warped_i = wp.tile([P, NX], F32)
            BL = 32
            for b0 in range(0, NX, BL):
                t8r = wp.tile([P, BL, 4], F32, tag="t8r", bufs=2)
                nc.vector.tensor_tensor(t8r[:], g8[:, b0:b0 + BL, 0:4],
                                        w4[:, b0:b0 + BL], op=ALU.mult)
                nc.vector.reduce_sum(warped_r[:, b0:b0 + BL],
                                     t8r[:], axis=mybir.AxisListType.X)
                t8i = wp.tile([P, BL, 4], F32, tag="t8i", bufs=2)
                nc.vector.tensor_tensor(t8i[:], g8[:, b0:b0 + BL, 4:8],
                                        w4[:, b0:b0 + BL], op=ALU.mult)
                nc.vector.reduce_sum(warped_i[:, b0:b0 + BL],
                                     t8i[:], axis=mybir.AxisListType.X)
            dbg_out("warped_r", warped_r[:], [P, NX])
            dbg_out("warped_i", warped_i[:], [P, NX])

            csm_r_sb = wp.tile([P, NCOIL, NX], F32)
            nc.sync.dma_start(csm_r_sb[:], csm_r.rearrange("c x y -> x c y"))
            csm_i_sb = wp.tile([P, NCOIL, NX], F32)
            nc.sync.dma_start(csm_i_sb[:], csm_i.rearrange("c x y -> x c y"))

            wr_b = warped_r[:].rearrange("p (c y) -> p c y", c=1).to_broadcast(
                [P, NCOIL, NX])
            wi_b = warped_i[:].rearrange("p (c y) -> p c y", c=1).to_broadcast(
                [P, NCOIL, NX])

            tt1 = wp.tile([P, NCOIL, NX], F32)
            nc.vector.tensor_tensor(tt1[:], csm_r_sb[:], wr_b, op=ALU.mult)
            tt2 = wp.tile([P, NCOIL, NX], F32)
            nc.gpsimd.tensor_tensor(tt2[:], csm_i_sb[:], wi_b, op=ALU.mult)
            coilr = wp.tile([P, NCOIL, NX], F32)
            nc.vector.tensor_tensor(coilr[:], tt1[:], tt2[:], op=ALU.subtract)
            tt3 = wp.tile([P, NCOIL, NX], F32)
            nc.gpsimd.tensor_tensor(tt3[:], csm_r_sb[:], wi_b, op=ALU.mult)
            tt4 = wp.tile([P, NCOIL, NX], F32)
            nc.vector.tensor_tensor(tt4[:], csm_i_sb[:], wr_b, op=ALU.mult)
            coili = wp.tile([P, NCOIL, NX], F32)
            nc.gpsimd.tensor_tensor(coili[:], tt3[:], tt4[:], op=ALU.add)
            dbg_out("coilr", coilr[:], [P, NCOIL, NX])
            dbg_out("coili", coili[:], [P, NCOIL, NX])

            # gr = (-coili)*kri + coilr*krr ; gi = coilr*kri + coili*krr
            def coil_view(t, h):
                return t[:].rearrange("p c (h yo1 yi) -> p yi h c yo1",
                                      h=2, yo1=YO2, yi=YI)[:, :, h]

            def pack_view(t, h, ri):
                return t[:, :, h, ri].rearrange("p yi (c yo1) -> p yi c yo1",
                                                c=NCOIL)

            for h in range(2):
                nc.vector.tensor_scalar(pack_view(RA, h, 0), coil_view(coili, h),
                                        -1.0, None, op0=ALU.mult)
                nc.gpsimd.tensor_copy(pack_view(RA, h, 1), coil_view(coilr, h))
                nc.vector.tensor_copy(pack_view(RB, h, 0), coil_view(coilr, h))
                nc.gpsimd.tensor_copy(pack_view(RB, h, 1), coil_view(coili, h))

        # ================ PE tail: chunk drain ================
        with tc.tile_pool(name="tp", bufs=1) as tp:
            for ch in range(NCHUNK):
                cp, half = divmod(ch, 2)
                sl = slice(half * CH, (half + 1) * CH)
                c0 = ch * CH

                mabsA = tp.tile([P, 2, CH], F32, tag="mabsA", bufs=2)
                nc.scalar.activation(mabsA[:], m2A[:, cp, :, sl], ACTF.Abs)
                aic = tp.tile([P, 2, CH], F32, tag="aic", bufs=2)
                nc.scalar.activation(aic[:], m2A[:, cp, :, sl], ACTF.Sin,
                                     scale=-TWO_PI)
                arc = tp.tile([P, 2, CH], F32, tag="arc", bufs=2)
                nc.scalar.activation(arc[:], mabsA[:], ACTF.Sin,
                                     scale=-TWO_PI, bias=half_pi[:, 0:1])

                gr = ps.tile([P, 2, CH], F32, tag="gr")
                gi = ps.tile([P, 2, CH], F32, tag="gi")
                for h in range(2):
                    for yi in range(YI):
                        nc.tensor.matmul(gr[:, h], RA[:, yi, h, 0],
                                         kri[:, cp, yi, sl],
                                         start=(yi == 0), stop=False)
                        nc.tensor.matmul(gr[:, h], RB[:, yi, h, 0],
                                         krr[:, cp, yi, sl],
                                         start=False, stop=(yi == YI - 1))
                    for yi in range(YI):
                        nc.tensor.matmul(gi[:, h], RA[:, yi, h, 1],
                                         kri[:, cp, yi, sl],
                                         start=(yi == 0), stop=False)
                        nc.tensor.matmul(gi[:, h], RB[:, yi, h, 1],
                                         krr[:, cp, yi, sl],
                                         start=False, stop=(yi == YI - 1))

                if ch == 0:
                    grs = tp.tile([P, 2, CH], F32, tag="dbg_gr")
                    nc.vector.tensor_copy(grs[:], gr[:])
                    dbg_out("gr0", grs[:], [P, 2, CH])

                pr = tp.tile([P, 2, CH], BF16, tag="pr")
                pi_ = tp.tile([P, 2, CH], BF16, tag="pi")
                t1 = tp.tile([P, 2, CH], F32, tag="t1")
                nc.vector.tensor_tensor(t1[:], gr[:], arc[:], op=ALU.mult)
                t2 = tp.tile([P, 2, CH], F32, tag="t2")
                nc.vector.tensor_tensor(t2[:], gi[:], aic[:], op=ALU.mult)
                nc.vector.tensor_tensor(pr[:], t1[:], t2[:], op=ALU.subtract)
                t3 = tp.tile([P, 2, CH], F32, tag="t3")
                nc.vector.tensor_tensor(t3[:], gi[:], arc[:], op=ALU.mult)
                t4 = tp.tile([P, 2, CH], F32, tag="t4")
                nc.vector.tensor_tensor(t4[:], gr[:], aic[:], op=ALU.mult)
                nc.vector.tensor_tensor(pi_[:], t3[:], t4[:], op=ALU.add)

                orps = pso.tile([NCOIL, 2, CH], F32, tag="osel")
                nc.tensor.matmul(orps[:, 0], sel[:], pr[:, 0], start=True,
                                 stop=False)
                nc.tensor.matmul(orps[:, 0], sel[:], pr[:, 1], start=False,
                                 stop=True)
                nc.tensor.matmul(orps[:, 1], sel[:], pi_[:, 0], start=True,
                                 stop=False)
                nc.tensor.matmul(orps[:, 1], sel[:], pi_[:, 1], start=False,
                                 stop=True)

                osr = tp.tile([NCOIL, 2, CH], F32, tag="osr", bufs=2)
                nc.scalar.copy(osr[:], orps[:])
                nc.sync.dma_start(out_r[:, c0:c0 + CH], osr[:, 0])
                nc.sync.dma_start(out_i[:, c0:c0 + CH], osr[:, 1])


_COMPILED = {}


def _get_nc():
    if "nc" not in _COMPILED:
        nc = bacc.Bacc("TRN2", debug=False)
        build_program(nc)
        nc.compile()
        _COMPILED["nc"] = nc
    return _COMPILED["nc"]


def make_in_maps(image_r, image_i, csm_r, csm_i, traj, dcf, flow):
    del dcf  # unused by the operator
    in_maps = []
    for core in range(8):
        t, h = divmod(core, 2)
        sl = slice(h * S, (h + 1) * S)
        in_maps.append({
            "image_r": np.ascontiguousarray(image_r, np.float32),
            "image_i": np.ascontiguousarray(image_i, np.float32),
            "csm_r": np.ascontiguousarray(csm_r, np.float32),
            "csm_i": np.ascontiguousarray(csm_i, np.float32),
            "kx": np.ascontiguousarray(traj[sl, 0, t], np.float32),
            "ky": np.ascontiguousarray(traj[sl, 1, t], np.float32),
            "flow0": np.ascontiguousarray(flow[:, :, 0, t], np.float32),
            "flow1": np.ascontiguousarray(flow[:, :, 1, t], np.float32),
        })
    return in_maps


def combine_outputs(results):
    out = np.zeros((NCOIL, NS), np.complex64)
    for core, res in enumerate(results):
        t, h = divmod(core, 2)
        sl = slice(h * S, (h + 1) * S)
        out[:, sl] += res["out_r"].astype(np.complex64) + 1j * res["out_i"].astype(
            np.complex64)
    return out


def kernel(**inputs) -> np.ndarray:
    from concourse.bass_utils import run_bass_kernel_spmd

    nc = _get_nc()
    in_maps = make_in_maps(**inputs)
    res = run_bass_kernel_spmd(nc, in_maps, core_ids=list(range(8)))
    return combine_outputs(res.results)
